# revision 1
# baseline (speedup 1.0000x reference)
"""Trainium2 Bass kernel for nn_DebiasLoss: data-parallel mean cross-entropy
with class-prior margin and target-column dispersion margin.

Sharding: logits/targets split along batch across 8 NeuronCores; w_norm /
class_bias replicated; each core emits (sum of its row losses)/B and the host
adds the 8 partial scalars (the all-reduce of the hint).

Math per row r (t = target logit rounded to bf16, BETA=0.5, LAMDA=1.0):
    E[c]   = exp(lt[c] - t)                  (ScalarE, bias = -t per partition;
                                              target term is exactly 1)
    S0     = sum_c cb[c] * E[c]              (DVE tensor_tensor_reduce or
                                              GpSimd scalar_tensor_tensor,
                                              f32 accumulator)
    cnt    = sum_c 1[E[c] > 1]               (DVE tensor_scalar, 4x bf16 mode)
    delta  = (cnt > 0) * BETA*coef * log1p((t32/w - w)^2)
    S_adj  = S0 + cb_t * (exp(-delta) - 1)
    loss_r = log(S_adj) - mlf_t + delta
which equals logsumexp(adj) - adj[t] of the reference (the -t shift cancels).

Host prep is data movement only: bf16 cast of logits, [B]-sized gathers of
the target logit / w_norm / mlf / class_bias tables, O(C) mlf precompute.
Logits stay in natural row order (no sort needed); per-row scalars ride in a
single small [P, 5T+1] table, so there are no on-device gathers at all.
"""

import os
from contextlib import ExitStack

import numpy as np
import ml_dtypes

B, C = 16384, 1000
N_CORES = 8
R = B // N_CORES  # 2048 rows per core
P = 128           # SBUF partitions
T = R // P        # 16 row-tiles per core
BETA = 0.5
LOG_EPS = 1e-12

# tabs column layout: [TGT | WN | MT | CBT | NEG | KBC]
NTAB = 5 * T + 1
C_TGT, C_WN, C_MT, C_CBT, C_NEG, C_KBC = 0, T, 2 * T, 3 * T, 4 * T, 5 * T


def _env_set(name, default):
    v = os.environ.get(name)
    if v is None:
        return set(default)
    v = v.strip()
    return set(int(x) for x in v.split(",") if x != "")


# Tiles where GpSimd computes W = E*cb (tensor_tensor, the only ALU op the
# Pool engine's ISA accepts) and DVE finishes with a 4x tensor_scalar accum;
# remaining tiles use a single DVE tensor_tensor_reduce.
GP_W = _env_set("KRN_GP_W", ())
# cnt pass on ScalarE as relu(lt - t) accum for these tiles (DVE 4x
# tensor_scalar is_gt for the rest)
SC_CNT = _env_set("KRN_SC_CNT", (1, 3, 5, 7, 9, 11, 13, 15))
# bisection aid: 2 = all-f32 DVE compute, proven op forms (STT, no TTR,
# no broadcast DMA); 1 = TTR but f32 operands; 0 = full bf16 fast path
SAFE = int(os.environ.get("KRN_SAFE", "2"))
# 1 = replace the whole [P,T] tail with a constant output write (crash bisect)
TAILTEST = int(os.environ.get("KRN_TAILTEST", "0"))

_CACHE = {}


def _patch_act_tables():
    """Make every activation this kernel uses resolve to the single table set
    natural_log_exp_and_others (Exp, Ln, Identity, Copy, ...), so the
    compiler emits one ACT_TABLE_LOAD instead of thrashing between sets."""
    import concourse.hw_specs as hw_specs
    import concourse.bacc as bacc_mod

    if _CACHE.get("tables_patched"):
        return
    orig = hw_specs.get_activation_tables

    def filtered(module_arch):
        import concourse.mybir as mybir

        tabs = {k: set(v) for k, v in orig(module_arch).items()}
        keep_set = "natural_log_exp_and_others"
        ours = {
            mybir.ActivationFunctionType.Exp,
            mybir.ActivationFunctionType.Ln,
            mybir.ActivationFunctionType.Relu,
            mybir.ActivationFunctionType.Identity,
            mybir.ActivationFunctionType.Copy,
            mybir.ActivationFunctionType.Square,
        }
        assert ours <= tabs[keep_set]
        for name, fns in tabs.items():
            if name != keep_set:
                tabs[name] = fns - ours
        return tabs

    hw_specs.get_activation_tables = filtered
    bacc_mod.get_activation_tables = filtered
    _CACHE["tables_patched"] = True


def _build(debug_taps=False):
    import concourse.bacc as bacc
    import concourse.bass as bass
    import concourse.tile as tile
    from concourse import mybir

    _patch_act_tables()

    f32 = mybir.dt.float32
    bf16 = mybir.dt.bfloat16
    Alu = mybir.AluOpType
    Act = mybir.ActivationFunctionType
    X = mybir.AxisListType.X

    nc = bacc.Bacc(
        "TRN2",
        target_bir_lowering=False,
        debug=False,
        enable_asserts=False,
        num_devices=N_CORES,
    )

    cdt = f32 if SAFE else bf16  # dtype of on-chip compute operands
    ldt = f32 if SAFE >= 3 else bf16
    d_logits = nc.dram_tensor("lgt", [R, C], ldt, kind="ExternalInput")
    d_tabs = nc.dram_tensor("tabs", [P, NTAB], f32, kind="ExternalInput")
    d_cb = nc.dram_tensor("cb_row", [1, C], cdt, kind="ExternalInput")
    d_out = nc.dram_tensor("out", [1, 1], f32, kind="ExternalOutput")
    d_dbg = {}
    if debug_taps:
        for nm in ("dbg_S0", "dbg_cnt", "dbg_delta", "dbg_lossr"):
            d_dbg[nm] = nc.dram_tensor(nm, [P, T], f32, kind="ExternalOutput")

    with tile.TileContext(nc) as tc:
        with ExitStack() as ctx:
            ltp = ctx.enter_context(tc.tile_pool(name="ltp", bufs=6))
            epp = ctx.enter_context(tc.tile_pool(name="epp", bufs=6))
            one = ctx.enter_context(tc.tile_pool(name="one", bufs=1))
            sm = ctx.enter_context(tc.tile_pool(name="sm", bufs=1))
            psp = ctx.enter_context(tc.tile_pool(name="psp", bufs=1, space="PSUM"))
            wgp = ctx.enter_context(tc.tile_pool(name="wgp", bufs=2))

            # ---- inputs ---------------------------------------------------
            tabs = sm.tile([P, NTAB], f32, tag="tabs")
            nc.sync.dma_start(out=tabs[:], in_=d_tabs.ap())
            lts = {}
            for j in range(min(T, 3)):
                lt_t = ltp.tile([P, C], ldt, tag="lt")
                nc.sync.dma_start(
                    out=lt_t[:], in_=d_logits.ap()[j * P : (j + 1) * P, :]
                )
                lts[j] = lt_t
            cb_bc = one.tile([P, C], cdt, tag="cb_bc")
            nc.sync.dma_start(out=cb_bc[:], in_=d_cb.ap().to_broadcast([P, C]))
            # standalone per-tile bias tiles (ACT bias reads a tile start, the
            # baseline-proven form) and a ones column for the cnt threshold
            negs = []
            for j in range(T):
                ng = sm.tile([P, 1], f32, tag=f"neg{j}")
                nc.gpsimd.tensor_copy(ng[:], tabs[:, C_NEG + j : C_NEG + j + 1])
                negs.append(ng)
            onec = sm.tile([P, 1], f32, tag="onec")
            nc.gpsimd.memset(onec[:], 1.0)
            zeroc = sm.tile([P, 1], f32, tag="zeroc")
            nc.gpsimd.memset(zeroc[:], 0.0)

            # ---- main loop over 16 row-tiles ------------------------------
            S0 = sm.tile([P, T], f32, tag="S0")
            cnt = sm.tile([P, T], f32, tag="cnt")
            garb_v = one.tile([P, C], cdt, tag="garb_v")
            garb_v2 = one.tile([P, C], cdt, tag="garb_v2")
            garb_a = one.tile([P, C], cdt, tag="garb_a")

            for j in range(T):
                if j not in lts:
                    lt_t = ltp.tile([P, C], ldt, tag="lt")
                    nc.sync.dma_start(
                        out=lt_t[:], in_=d_logits.ap()[j * P : (j + 1) * P, :]
                    )
                    lts[j] = lt_t
                ep = epp.tile([P, C], cdt, tag="ep")
                nc.scalar.activation(
                    out=ep[:], in_=lts[j][:], func=Act.Exp, bias=negs[j][:],
                )
                if j in GP_W:
                    wg = wgp.tile([P, C], cdt, tag="wg")
                    nc.gpsimd.tensor_tensor(
                        out=wg[:], in0=ep[:], in1=cb_bc[:], op=Alu.mult
                    )
                    nc.vector.tensor_scalar(
                        out=garb_v[:], in0=wg[:], scalar1=1.0, scalar2=None,
                        op0=Alu.mult, op1=Alu.add,
                        accum_out=S0[:, j : j + 1],
                    )
                elif SAFE >= 2:
                    nc.vector.scalar_tensor_tensor(
                        out=garb_v[:], in0=ep[:], scalar=zeroc[:, 0:1],
                        in1=cb_bc[:], op0=Alu.add, op1=Alu.mult,
                        accum_out=S0[:, j : j + 1],
                    )
                else:
                    nc.vector.tensor_tensor_reduce(
                        out=garb_v[:], in0=ep[:], in1=cb_bc[:], scale=1.0,
                        scalar=zeroc[:, 0:1], op0=Alu.mult, op1=Alu.add,
                        accum_out=S0[:, j : j + 1],
                    )
                if j in SC_CNT:
                    nc.scalar.activation(
                        out=garb_a[:], in_=lts[j][:], func=Act.Relu,
                        bias=negs[j][:], accum_out=cnt[:, j : j + 1],
                    )
                else:
                    nc.vector.tensor_scalar(
                        out=garb_v2[:], in0=ep[:], scalar1=onec[:, 0:1],
                        scalar2=None, op0=Alu.is_gt, op1=Alu.add,
                        accum_out=cnt[:, j : j + 1],
                    )

            # ---- per-row tail on [P, T] tiles -----------------------------
            # dispersion d0 = log1p((t/w - w)^2), from host tables only
            rw = sm.tile([P, T], f32, tag="rw")
            nc.vector.reciprocal(rw[:], tabs[:, C_WN : C_WN + T])
            t1 = sm.tile([P, T], f32, tag="t1")
            nc.vector.tensor_mul(t1[:], tabs[:, C_TGT : C_TGT + T], rw[:])
            q = sm.tile([P, T], f32, tag="q")
            nc.vector.tensor_tensor(
                out=q[:], in0=t1[:], in1=tabs[:, C_WN : C_WN + T], op=Alu.subtract
            )
            qq = sm.tile([P, T], f32, tag="qq")
            nc.vector.tensor_mul(qq[:], q[:], q[:])
            d0 = sm.tile([P, T], f32, tag="d0")
            nc.scalar.activation(out=d0[:], in_=qq[:], func=Act.Ln, bias=1.0)

            # delta = (cnt > 0) * beta*coef * d0
            kc = sm.tile([P, T], f32, tag="kc")
            nc.vector.tensor_scalar(
                out=kc[:], in0=cnt[:], scalar1=0.0,
                scalar2=tabs[:, C_KBC : C_KBC + 1],
                op0=Alu.is_gt, op1=Alu.mult,
            )
            delta = sm.tile([P, T], f32, tag="delta")
            nc.vector.tensor_mul(delta[:], kc[:], d0[:])

            # S_adj = S0 + cb_t*(exp(-delta) - 1);  loss_r = log(S_adj) - mlf_t + delta
            emd = sm.tile([P, T], f32, tag="emd")
            nc.scalar.activation(out=emd[:], in_=delta[:], func=Act.Exp, scale=-1.0)
            w_ = sm.tile([P, T], f32, tag="w_")
            nc.vector.scalar_tensor_tensor(
                out=w_[:], in0=emd[:], scalar=1.0,
                in1=tabs[:, C_CBT : C_CBT + T],
                op0=Alu.subtract, op1=Alu.mult,
            )
            sadj = sm.tile([P, T], f32, tag="sadj")
            nc.vector.tensor_tensor(out=sadj[:], in0=S0[:], in1=w_[:], op=Alu.add)
            lse = sm.tile([P, T], f32, tag="lse")
            nc.scalar.activation(out=lse[:], in_=sadj[:], func=Act.Ln)
            a1 = sm.tile([P, T], f32, tag="a1")
            nc.vector.tensor_tensor(out=a1[:], in0=lse[:], in1=delta[:], op=Alu.add)
            lossr = sm.tile([P, T], f32, tag="lossr")
            nc.vector.tensor_tensor(
                out=lossr[:], in0=a1[:], in1=tabs[:, C_MT : C_MT + T],
                op=Alu.subtract,
            )

            # ---- reduce 2048 row losses to one scalar ---------------------
            rowsum = sm.tile([P, 1], f32, tag="rowsum")
            nc.vector.tensor_reduce(rowsum[:], lossr[:], axis=X, op=Alu.add)
            invb = sm.tile([P, 1], f32, tag="invb")
            nc.vector.memset(invb[:], 1.0 / B)
            ps = psp.tile([1, 1], f32, tag="ps")
            nc.tensor.matmul(out=ps[:], lhsT=rowsum[:], rhs=invb[:], start=True, stop=True)
            res = sm.tile([1, 1], f32, tag="res")
            nc.vector.tensor_copy(res[:], ps[:])
            nc.sync.dma_start(out=d_out.ap(), in_=res[:])

            if debug_taps:
                for nm, tl in [
                    ("dbg_S0", S0), ("dbg_cnt", cnt), ("dbg_delta", delta),
                    ("dbg_lossr", lossr),
                ]:
                    nc.sync.dma_start(out=d_dbg[nm].ap(), in_=tl[:])

    nc.compile()
    return nc


def _get_nc(debug_taps=False):
    key = "nc_dbg" if debug_taps else "nc"
    if key not in _CACHE:
        _CACHE[key] = _build(debug_taps=debug_taps)
    return _CACHE[key]


def _prep_in_maps(logits, targets, adaptive_marg_coef, w_norm, class_bias):
    logits = np.asarray(logits, dtype=np.float32)
    assert logits.shape == (B, C), logits.shape
    t = np.asarray(targets).astype(np.int64).ravel()
    w = np.asarray(w_norm, dtype=np.float32).ravel()
    cb = np.asarray(class_bias, dtype=np.float32).ravel()
    coef = float(np.asarray(adaptive_marg_coef, dtype=np.float32).reshape(()))

    mlf = np.log(cb + LOG_EPS)
    cb_dt = np.float32 if SAFE else ml_dtypes.bfloat16
    cb_row = np.ascontiguousarray(cb.reshape(1, C)).astype(cb_dt)

    in_maps = []
    for k in range(N_CORES):
        sl = slice(k * R, (k + 1) * R)
        lg = logits[sl]
        ts = t[sl]
        lt_bf = lg.astype(ml_dtypes.bfloat16)
        rows = np.arange(R)
        tgt32 = lg[rows, ts]                              # exact target logit
        tgtb = lt_bf[rows, ts].astype(np.float32)         # bf16-rounded
        tabs = np.empty((P, NTAB), dtype=np.float32)

        def col(v):  # row r = 128*j + p  ->  [P, T] with column j = tile j
            return v.reshape(T, P).T

        tabs[:, C_TGT : C_TGT + T] = col(tgt32)
        tabs[:, C_WN : C_WN + T] = col(w[ts])
        tabs[:, C_MT : C_MT + T] = col(mlf[ts])
        tabs[:, C_CBT : C_CBT + T] = col(cb[ts] + LOG_EPS)
        tabs[:, C_NEG : C_NEG + T] = col(-tgtb)
        tabs[:, C_KBC] = BETA * coef
        in_maps.append(
            {
                "lgt": np.ascontiguousarray(lt_bf if SAFE < 3 else lg),
                "tabs": tabs,
                "cb_row": cb_row,
            }
        )
    return in_maps


def _run(inputs, trace=False, debug_taps=False):
    from concourse import bass_utils

    in_maps = _prep_in_maps(**inputs)
    nc = _get_nc(debug_taps=debug_taps)
    res = bass_utils.run_bass_kernel_spmd(
        nc, in_maps, core_ids=list(range(N_CORES)), trace=trace
    )
    total = sum(float(r["out"][0, 0]) for r in res.results)
    return np.float32(total), res


def kernel(**inputs) -> np.ndarray:
    loss, _ = _run(inputs, trace=False)
    return loss

